# revision 6
# baseline (speedup 1.0000x reference)
"""CBOW negative-sampling loss kernel for Trainium2 (8 NeuronCores, SPMD).

Per batch element b: gather 21 rows of 50 floats (10 ctx rows from in_embed,
1 pos + 10 neg from out_embed), context sum, 11 dot products, log-sigmoids,
global mean.

This runtime's indirect DMA consumes ONE offset per partition per op
(HW-verified: multi-offset APs silently use only offset[p, 0] and fetch a
contiguous block), so the kernel issues one indirect_dma_start per
(tile, j): a [128,1] offset column gathers one table row per partition.
21 gathers per 128-element tile, 2688 per core.

Optimizations over the v0 baseline (145.6us):
- Table stored fp8e4m3 with rows padded to 64B (exactly one aligned HBM
  burst per row, half the random-read traffic of fp16) and cast to fp16
  by the DMA during the gather (HW-verified).  Dest rows are 52 elems
  (50 + 2 zero pads from the table padding) so fold halves stay
  4B-aligned for the DVE 2x perf mode.
- Compute batched over groups of 8 tiles: 4 tree adds for the ctx sum,
  one broadcast mul (stride-0 AP over the 11 out-rows), one 52->26 fold
  add, one 26->1 reduce - a few large tensor ops instead of many small
  ones (~180 DVE instructions/core vs ~2180 in the baseline).
- pos/neg sign and the /10 context mean are folded into two strided
  sigmoid activations (scale +-0.1); Ln(+eps) with accum_out produces
  the per-partition loss sums in one pass.
Host: loss = -(sum of partials) / B.
"""

import sys

import numpy as np

if "/opt/trn_rl_repo" not in sys.path:
    sys.path.insert(0, "/opt/trn_rl_repo")

from concourse import bass, mybir  # noqa: E402
from concourse import bass_utils  # noqa: E402
from concourse import tile  # noqa: E402
from concourse.bacc import Bacc  # noqa: E402

VOCAB = 50000
DIM = 50
B = 131072
CTX = 10
NEG = 10
NIDX = CTX + 1 + NEG  # 21 rows per batch element: [ctx*10, pos, neg*10]
EPS = 1e-10

NCORES = 8
P = 128
BC = B // NCORES  # 16384
NTILES = BC // P  # 128
TW = 64  # table row width (fp8 bytes, one HBM burst)
DP = 52  # gathered row width in SBUF (50 data + 2 zero pads)
GT = 16  # tiles per compute group

f8 = mybir.dt.float8e4
f16 = mybir.dt.float16
f32 = mybir.dt.float32
i32 = mybir.dt.int32


def build_nc(ntiles: int = NTILES, repeats: int = 1, dump_scores: bool = False):
    nc = Bacc(None, target_bir_lowering=False)
    eps_t = nc.alloc_sbuf_tensor("const-eps", [P, 1], f32)
    nc.gpsimd.memset(eps_t.ap(), EPS)
    nc.const_aps.aps[(f32, EPS)] = eps_t.ap()
    nc.all_engine_barrier()

    table = nc.dram_tensor("table", [2 * VOCAB, TW], f8, kind="ExternalInput")
    idx = nc.dram_tensor(
        "idx", [P, ntiles * NIDX], mybir.dt.int32, kind="ExternalInput"
    )
    partial = nc.dram_tensor("partial", [P, 1], f32, kind="ExternalOutput")
    scores_out = (
        nc.dram_tensor("scores_out", [P, ntiles * 11], f32, kind="ExternalOutput")
        if dump_scores
        else None
    )

    ngroups = ntiles // GT
    assert ngroups * GT == ntiles

    with tile.TileContext(nc) as tc:
        with (
            tc.tile_pool(name="idxp", bufs=1) as ipool,
            tc.tile_pool(name="gather", bufs=2) as gpool,
            tc.tile_pool(name="work", bufs=1) as wpool,
            tc.tile_pool(name="stage", bufs=1) as spool,
        ):
          for rep in range(repeats):
            it = ipool.tile([P, ntiles * NIDX], i32, tag="it")
            nc.sync.dma_start(out=it[:], in_=idx[:])
            itv = it[:].rearrange("p (t j) -> p t j", t=ntiles, j=NIDX)

            scores = spool.tile([P, ntiles * 11], f32, tag="scores")
            scv = scores[:].rearrange(
                "p (g t j) -> p g t j", g=ngroups, t=GT, j=11
            )

            for g in range(ngroups):
                gt = gpool.tile([P, GT * NIDX * DP], f16, tag="g")
                gv = gt[:].rearrange(
                    "p (t j d) -> p t j d", t=GT, j=NIDX, d=DP
                )
                for ti in range(GT):
                    t = g * GT + ti
                    for j in range(NIDX):
                        nc.gpsimd.indirect_dma_start(
                            out=gv[:, ti, j, :],
                            out_offset=None,
                            in_=table[:],
                            in_offset=bass.IndirectOffsetOnAxis(
                                ap=itv[:, t, j : j + 1], axis=0
                            ),
                        )
                # context tree-sum over rows 0..9 -> ctx [P, GT, DP]
                s1 = wpool.tile([P, GT * 5 * DP], f16, tag="s1")
                s1v = s1[:].rearrange("p (t k d) -> p t k d", t=GT, k=5, d=DP)
                nc.vector.tensor_add(
                    out=s1v, in0=gv[:, :, 0:5, :], in1=gv[:, :, 5:10, :]
                )
                s2 = wpool.tile([P, GT * 2 * DP], f16, tag="s2")
                s2v = s2[:].rearrange("p (t k d) -> p t k d", t=GT, k=2, d=DP)
                nc.vector.tensor_add(
                    out=s2v, in0=s1v[:, :, 0:2, :], in1=s1v[:, :, 2:4, :]
                )
                s3 = wpool.tile([P, GT * DP], f16, tag="s3")
                s3v = s3[:].rearrange("p (t d) -> p t d", t=GT, d=DP)
                nc.vector.tensor_add(
                    out=s3v, in0=s2v[:, :, 0, :], in1=s2v[:, :, 1, :]
                )
                ctx = wpool.tile([P, GT * DP], f16, tag="ctx")
                ctxv = ctx[:].rearrange("p (t d) -> p t d", t=GT, d=DP)
                nc.vector.tensor_add(
                    out=ctxv, in0=s3v, in1=s1v[:, :, 4, :]
                )

                # products for rows 10..20 ([pos, neg*10])
                prod = wpool.tile([P, GT * 11 * DP], f16, tag="prod")
                prodv = prod[:].rearrange(
                    "p (t j d) -> p t j d", t=GT, j=11, d=DP
                )
                ctxb = ctxv.unsqueeze(2).broadcast_to((P, GT, 11, DP))
                nc.vector.tensor_mul(
                    out=prodv, in0=gv[:, :, 10:21, :], in1=ctxb
                )
                # fold 52 -> 26, then reduce 26 -> 1 (fp32 out)
                fb = wpool.tile([P, GT * 11 * 26], f16, tag="fb")
                fbv = fb[:].rearrange(
                    "p (t j d) -> p t j d", t=GT, j=11, d=26
                )
                nc.vector.tensor_add(
                    out=fbv, in0=prodv[:, :, :, 0:26], in1=prodv[:, :, :, 26:52]
                )
                nc.vector.tensor_reduce(
                    out=scv[:, g, :, :],
                    in_=fbv,
                    axis=mybir.AxisListType.X,
                    op=mybir.AluOpType.add,
                    negate=False,
                )

            acc = spool.tile([P, 1], f32, tag="acc")
            if dump_scores:
                nc.sync.dma_start(out=scores_out[:], in_=scores[:])
            sall = scores[:].rearrange("p (t j) -> p t j", t=ntiles, j=11)
            # pos slots: sigmoid(+0.1 * s);  neg slots: sigmoid(-0.1 * s)
            nc.scalar.activation(
                out=sall[:, :, 0:1],
                in_=sall[:, :, 0:1],
                func=mybir.ActivationFunctionType.Sigmoid,
                scale=0.1,
            )
            nc.scalar.activation(
                out=sall[:, :, 1:11],
                in_=sall[:, :, 1:11],
                func=mybir.ActivationFunctionType.Sigmoid,
                scale=-0.1,
            )
            nc.scalar.activation(
                out=scores[:],
                in_=scores[:],
                func=mybir.ActivationFunctionType.Ln,
                bias=EPS,
                accum_out=acc[:],
            )
            nc.sync.dma_start(out=partial[:], in_=acc[:])

    nc.compile()
    return nc


def _prep_inputs(context_idxs, pos_target, neg_samples, in_embed_W, out_embed_W):
    idx_all = np.concatenate(
        [
            np.asarray(context_idxs, dtype=np.int64),
            np.asarray(pos_target, dtype=np.int64)[:, None] + VOCAB,
            np.asarray(neg_samples, dtype=np.int64) + VOCAB,
        ],
        axis=1,
    ).astype(np.int32)  # [B, 21] = [ctx*10, pos, neg*10]

    table = np.zeros((2 * VOCAB, TW), dtype=mybir.dt.np(f8))
    table[:VOCAB, :DIM] = np.asarray(in_embed_W).astype(mybir.dt.np(f8))
    table[VOCAB:, :DIM] = np.asarray(out_embed_W).astype(mybir.dt.np(f8))

    in_maps = []
    for c in range(NCORES):
        sl = idx_all[c * BC : (c + 1) * BC]
        idx_c = (
            sl.reshape(NTILES, P, NIDX)
            .transpose(1, 0, 2)
            .reshape(P, NTILES * NIDX)
            .copy()
        )
        in_maps.append({"table": table, "idx": idx_c})
    return in_maps


def kernel(context_idxs, pos_target, neg_samples, in_embed_W, out_embed_W):
    in_maps = _prep_inputs(
        context_idxs, pos_target, neg_samples, in_embed_W, out_embed_W
    )
    nc = build_nc()
    res = bass_utils.run_bass_kernel_spmd(nc, in_maps, core_ids=list(range(NCORES)))
    total = sum(float(r["partial"].sum()) for r in res.results)
    return np.float32(-total / B)


# revision 7
# speedup vs baseline: 1.0290x; 1.0290x over previous
"""CBOW negative-sampling loss kernel for Trainium2 (8 NeuronCores, SPMD).

Per batch element b: gather 21 rows of 50 floats (10 ctx rows from in_embed,
1 pos + 10 neg from out_embed), context sum, 11 dot products, log-sigmoids,
global mean.

This runtime's indirect DMA consumes ONE offset per partition per op
(HW-verified: multi-offset APs silently use only offset[p, 0] and fetch a
contiguous block), so the kernel issues one indirect_dma_start per
(tile, j): a [128,1] offset column gathers one table row per partition.
21 gathers per 128-element tile, 2688 per core.

Optimizations over the v0 baseline (145.6us):
- Table stored fp8e4m3 with rows padded to 64B (exactly one aligned HBM
  burst per row, half the random-read traffic of fp16) and cast to fp16
  by the DMA during the gather (HW-verified).  Dest rows are 52 elems
  (50 + 2 zero pads from the table padding) so fold halves stay
  4B-aligned for the DVE 2x perf mode.
- Compute batched over groups of 16 tiles: 4 tree adds for the ctx sum,
  one broadcast mul (stride-0 AP over the 11 out-rows), one 52->26 fold
  add, one 26->1 reduce - a few large tensor ops instead of many small
  ones (56 DVE compute instructions/core vs ~2180 in the baseline).
- pos/neg sign and the /10 context mean are folded into two strided
  sigmoid activations (scale +-0.1); Ln(+eps) with accum_out produces
  the per-partition loss sums in one pass.
Host: loss = -(sum of partials) / B.
"""

import sys

import numpy as np

if "/opt/trn_rl_repo" not in sys.path:
    sys.path.insert(0, "/opt/trn_rl_repo")

from concourse import bass, mybir  # noqa: E402
from concourse import bass_utils  # noqa: E402
from concourse import tile  # noqa: E402
from concourse.bacc import Bacc  # noqa: E402

VOCAB = 50000
DIM = 50
B = 131072
CTX = 10
NEG = 10
NIDX = CTX + 1 + NEG  # 21 rows per batch element: [ctx*10, pos, neg*10]
EPS = 1e-10

NCORES = 8
P = 128
BC = B // NCORES  # 16384
NTILES = BC // P  # 128
TW = 64  # table row width (fp8 bytes, one HBM burst)
DP = 52  # gathered row width in SBUF (50 data + 2 zero pads)
GT = 16  # tiles per compute group

f8 = mybir.dt.float8e4
f16 = mybir.dt.float16
f32 = mybir.dt.float32
i32 = mybir.dt.int32


def build_nc(ntiles: int = NTILES, repeats: int = 1, dump_scores: bool = False):
    nc = Bacc(None, target_bir_lowering=False)
    eps_t = nc.alloc_sbuf_tensor("const-eps", [P, 1], f32)
    nc.gpsimd.memset(eps_t.ap(), EPS)
    nc.const_aps.aps[(f32, EPS)] = eps_t.ap()
    nc.all_engine_barrier()

    table = nc.dram_tensor("table", [2 * VOCAB, TW], f8, kind="ExternalInput")
    idx = nc.dram_tensor(
        "idx", [P, ntiles * NIDX], mybir.dt.int32, kind="ExternalInput"
    )
    partial = nc.dram_tensor("partial", [P, 1], f32, kind="ExternalOutput")
    scores_out = (
        nc.dram_tensor("scores_out", [P, ntiles * 11], f32, kind="ExternalOutput")
        if dump_scores
        else None
    )

    ngroups = ntiles // GT
    assert ngroups * GT == ntiles

    with tile.TileContext(nc) as tc:
        with (
            tc.tile_pool(name="idxp", bufs=1) as ipool,
            tc.tile_pool(name="gather", bufs=2) as gpool,
            tc.tile_pool(name="work", bufs=1) as wpool,
            tc.tile_pool(name="stage", bufs=1) as spool,
        ):
          for rep in range(repeats):
            it = ipool.tile([P, ntiles * NIDX], i32, tag="it")
            nc.sync.dma_start(out=it[:], in_=idx[:])
            itv = it[:].rearrange("p (t j) -> p t j", t=ntiles, j=NIDX)

            scores = spool.tile([P, ntiles * 11], f32, tag="scores")
            scv = scores[:].rearrange(
                "p (g t j) -> p g t j", g=ngroups, t=GT, j=11
            )

            for g in range(ngroups):
                gt = gpool.tile([P, GT * NIDX * DP], f16, tag="g")
                gv = gt[:].rearrange(
                    "p (t j d) -> p t j d", t=GT, j=NIDX, d=DP
                )
                for ti in range(GT):
                    t = g * GT + ti
                    for j in range(NIDX):
                        nc.gpsimd.indirect_dma_start(
                            out=gv[:, ti, j, :],
                            out_offset=None,
                            in_=table[:],
                            in_offset=bass.IndirectOffsetOnAxis(
                                ap=itv[:, t, j : j + 1], axis=0
                            ),
                        )
                # context tree-sum over rows 0..9 -> ctx [P, GT, DP]
                s1 = wpool.tile([P, GT * 5 * DP], f16, tag="s1")
                s1v = s1[:].rearrange("p (t k d) -> p t k d", t=GT, k=5, d=DP)
                nc.vector.tensor_add(
                    out=s1v, in0=gv[:, :, 0:5, :], in1=gv[:, :, 5:10, :]
                )
                s2 = wpool.tile([P, GT * 2 * DP], f16, tag="s2")
                s2v = s2[:].rearrange("p (t k d) -> p t k d", t=GT, k=2, d=DP)
                nc.vector.tensor_add(
                    out=s2v, in0=s1v[:, :, 0:2, :], in1=s1v[:, :, 2:4, :]
                )
                s3 = wpool.tile([P, GT * DP], f16, tag="s3")
                s3v = s3[:].rearrange("p (t d) -> p t d", t=GT, d=DP)
                nc.vector.tensor_add(
                    out=s3v, in0=s2v[:, :, 0, :], in1=s2v[:, :, 1, :]
                )
                ctx = wpool.tile([P, GT * DP], f16, tag="ctx")
                ctxv = ctx[:].rearrange("p (t d) -> p t d", t=GT, d=DP)
                nc.vector.tensor_add(
                    out=ctxv, in0=s3v, in1=s1v[:, :, 4, :]
                )

                # products for rows 10..20 ([pos, neg*10])
                prod = wpool.tile([P, GT * 11 * DP], f16, tag="prod")
                prodv = prod[:].rearrange(
                    "p (t j d) -> p t j d", t=GT, j=11, d=DP
                )
                ctxb = ctxv.unsqueeze(2).broadcast_to((P, GT, 11, DP))
                nc.vector.tensor_mul(
                    out=prodv, in0=gv[:, :, 10:21, :], in1=ctxb
                )
                # fold 52 -> 26, then reduce 26 -> 1 (fp32 out)
                fb = wpool.tile([P, GT * 11 * 26], f16, tag="fb")
                fbv = fb[:].rearrange(
                    "p (t j d) -> p t j d", t=GT, j=11, d=26
                )
                nc.vector.tensor_add(
                    out=fbv, in0=prodv[:, :, :, 0:26], in1=prodv[:, :, :, 26:52]
                )
                nc.vector.tensor_reduce(
                    out=scv[:, g, :, :],
                    in_=fbv,
                    axis=mybir.AxisListType.X,
                    op=mybir.AluOpType.add,
                    negate=False,
                )

            acc = spool.tile([P, 1], f32, tag="acc")
            if dump_scores:
                nc.sync.dma_start(out=scores_out[:], in_=scores[:])
            sall = scores[:].rearrange("p (t j) -> p t j", t=ntiles, j=11)
            # pos slots: sigmoid(+0.1 * s);  neg slots: sigmoid(-0.1 * s)
            nc.scalar.activation(
                out=sall[:, :, 0:1],
                in_=sall[:, :, 0:1],
                func=mybir.ActivationFunctionType.Sigmoid,
                scale=0.1,
            )
            nc.scalar.activation(
                out=sall[:, :, 1:11],
                in_=sall[:, :, 1:11],
                func=mybir.ActivationFunctionType.Sigmoid,
                scale=-0.1,
            )
            nc.scalar.activation(
                out=scores[:],
                in_=scores[:],
                func=mybir.ActivationFunctionType.Ln,
                bias=EPS,
                accum_out=acc[:],
            )
            nc.sync.dma_start(out=partial[:], in_=acc[:])

    nc.compile()
    return nc


def _prep_inputs(context_idxs, pos_target, neg_samples, in_embed_W, out_embed_W):
    idx_all = np.concatenate(
        [
            np.asarray(context_idxs, dtype=np.int64),
            np.asarray(pos_target, dtype=np.int64)[:, None] + VOCAB,
            np.asarray(neg_samples, dtype=np.int64) + VOCAB,
        ],
        axis=1,
    ).astype(np.int32)  # [B, 21] = [ctx*10, pos, neg*10]

    table = np.zeros((2 * VOCAB, TW), dtype=mybir.dt.np(f8))
    table[:VOCAB, :DIM] = np.asarray(in_embed_W).astype(mybir.dt.np(f8))
    table[VOCAB:, :DIM] = np.asarray(out_embed_W).astype(mybir.dt.np(f8))

    in_maps = []
    for c in range(NCORES):
        sl = idx_all[c * BC : (c + 1) * BC]
        idx_c = (
            sl.reshape(NTILES, P, NIDX)
            .transpose(1, 0, 2)
            .reshape(P, NTILES * NIDX)
            .copy()
        )
        in_maps.append({"table": table, "idx": idx_c})
    return in_maps


def kernel(context_idxs, pos_target, neg_samples, in_embed_W, out_embed_W):
    in_maps = _prep_inputs(
        context_idxs, pos_target, neg_samples, in_embed_W, out_embed_W
    )
    nc = build_nc()
    res = bass_utils.run_bass_kernel_spmd(nc, in_maps, core_ids=list(range(NCORES)))
    total = sum(float(r["partial"].sum()) for r in res.results)
    return np.float32(-total / B)
